# revision 1
# baseline (speedup 1.0000x reference)
"""Trainium2 Bass kernel for nn_DistributionLossWithLabel.

Reference computation (B=8192, C=64):
    lq = log(q); lp = log(p)
    positive[i] = mean_c p[i,c]*(lp[i,c]-lq[i,c])
    a[j]        = sum_c p[j,c]*lp[j,c] / C
    kl[i,j]     = a[j] - (lq @ p^T)[i,j] / C
    negative[i] = sum_j kl[i,j] + sum_j kl[i,j]*(1-L[i,j])
    loss        = sum_i positive[i]/negative[i]

Device reformulation (rows i sharded 8 ways, D = 2 - L shipped from host
transposed as bf16; {1,2} and {0,1} are exact in bf16):
    negative[i] = sum_j kl[i,j]*(2-L[i,j])
                = (D@a)[i] - sum_c (lq[i,c]/C) * (D@p)[i,c]
    [Dp | Da] accumulates on the TensorEngine as paug^T @ D^T where
    paug = [p | a_hi | a_lo] (bf16, with a carried as a hi/lo split to
    kill the bf16 rounding of the dominant term), streamed against D^T
    tiles straight from HBM.  The 8192x8192 KL matrix never exists, the
    VectorEngine only does O(B) epilogue work, and the kernel is bound by
    reading D^T once (16MB/core).
"""

import sys

if "/opt/trn_rl_repo" not in sys.path:
    sys.path.insert(0, "/opt/trn_rl_repo")

import ml_dtypes
import numpy as np

import concourse.bass as bass
import concourse.tile as tile
from concourse import bacc, mybir
from concourse.masks import make_identity

FP = mybir.dt.float32
BF = mybir.dt.bfloat16
F8 = mybir.dt.float8e4
AF = mybir.ActivationFunctionType
ALU = mybir.AluOpType
AX = mybir.AxisListType

B_FULL = 8192
C = 64
N_CORES = 8
NAUG = 66  # 64 p columns + a_hi + a_lo


def build_nc(B=B_FULL, shard=B_FULL // N_CORES, debug=False):
    """Build the single-core SPMD Bass program.

    B: total rows (j extent), multiple of 512.
    shard: rows per core (i extent), multiple of 128.
    """
    assert B % 512 == 0 and shard % 128 == 0
    njc = B // 128           # 128-row j-chunks of p / D^T
    nblk = shard // 128      # 128-row i-blocks
    nhalf = (shard + 511) // 512
    ccpt = 4                 # j-chunks per D^T DMA tile
    assert njc % ccpt == 0
    rcpC = 1.0 / C

    nc = bacc.Bacc("TRN2", target_bir_lowering=False, debug=debug)

    # q/p/p_my arrive pre-chunked from host: [128, nchunks*64] where
    # partition pp, col n*64+c = row n*128+pp, col c — so every input DMA
    # is contiguous per partition (line rate) and rows land on partitions.
    q_d = nc.dram_tensor("q", [128, nblk * 64], FP, kind="ExternalInput")
    p_d = nc.dram_tensor("p", [128, njc * 64], FP, kind="ExternalInput")
    pmy_d = nc.dram_tensor("p_my", [128, nblk * 64], FP, kind="ExternalInput")
    # D^T = (2 - labels)^T for this core's row shard: [B, shard] fp8e4m3
    # ({1,2} are exact in e4m3; the PE takes bf16 weights x fp8 moving)
    lab_d = nc.dram_tensor("labels", [B, shard], F8, kind="ExternalInput")
    out_d = nc.dram_tensor("out", [128, 1], FP, kind="ExternalOutput")

    with tile.TileContext(nc) as tc:
        with (
            tc.tile_pool(name="const", bufs=1) as cp,
            tc.tile_pool(name="lpool", bufs=8) as lp_pool,
            tc.tile_pool(name="spool", bufs=2) as sp,
            tc.tile_pool(name="mps_ps", bufs=1, space="PSUM") as mps_ps,
            tc.tile_pool(name="tr_ps", bufs=2, space="PSUM") as tr_ps,
        ):
            ident = cp.tile([128, 128], FP)
            make_identity(nc, ident[:])

            # ---------------- p prologue -> paug (pipelined quarters) -------
            # Quarter-granular ops + subtile deps let main-loop matmuls on
            # early chunks start while later quarters are still loading.
            P_nat = cp.tile([128, njc * 64], FP)
            LP = cp.tile([128, njc * 64], FP)
            A = cp.tile([128, njc * 64], FP)
            asum = cp.tile([128, njc], FP)  # sum_c p*lp (unscaled)
            ah32 = cp.tile([128, njc], FP)
            alo = cp.tile([128, njc], FP)
            paug = cp.tile([128, njc * NAUG], BF)
            paug_v = paug[:].rearrange("p (n w) -> p n w", w=NAUG)

            # First D^T tile on the fast HWDGE ring before anything else —
            # the first matmuls need it and SWDGE has a slow ramp.
            lab_ap = lab_d.ap()
            Lt0 = lp_pool.tile([128, ccpt, shard], F8, tag="L")
            nc.sync.dma_start(
                out=Lt0[:],
                in_=lab_ap[0 : ccpt * 128, :].rearrange("(cc p) i -> p cc i", p=128),
            )

            NQ = 8
            qw = njc // NQ
            p_ap = p_d.ap()
            for qd in range(NQ):
                ns = slice(qd * qw, (qd + 1) * qw)
                fs = slice(qd * qw * 64, (qd + 1) * qw * 64)
                nc.sync.dma_start(out=P_nat[:, fs], in_=p_ap[:, fs])
                nc.scalar.activation(LP[:, fs], P_nat[:, fs], AF.Ln)
                nc.vector.tensor_tensor(
                    A[:, fs], P_nat[:, fs], LP[:, fs], op=ALU.mult
                )
                nc.vector.reduce_sum(
                    asum[:, ns],
                    A[:, fs].rearrange("p (n c) -> p n c", c=64),
                    axis=AX.X,
                )
                nc.scalar.copy(
                    paug_v[:, ns, 0:64],
                    P_nat[:, fs].rearrange("p (n c) -> p n c", c=64),
                )
                # a_hi = bf16(a), a_lo = bf16(a - a_hi); a = asum/C
                nc.scalar.activation(
                    paug_v[:, ns, 64:65],
                    asum[:, ns].rearrange("p (n o) -> p n o", o=1),
                    AF.Copy,
                    scale=rcpC,
                )
                nc.vector.tensor_copy(
                    ah32[:, ns].rearrange("p (n o) -> p n o", o=1),
                    paug_v[:, ns, 64:65],
                )
                nc.vector.scalar_tensor_tensor(
                    out=alo[:, ns],
                    in0=asum[:, ns],
                    scalar=rcpC,
                    in1=ah32[:, ns],
                    op0=ALU.mult,
                    op1=ALU.subtract,
                )
                nc.scalar.copy(
                    paug_v[:, ns, 65:66],
                    alo[:, ns].rearrange("p (n o) -> p n o", o=1),
                )

            # ---------------- main loop: [Dp|Da]^T += paug^T @ D^T ----------
            mps = mps_ps.tile([128, shard], FP)
            for g in range(njc // ccpt):
                if g == 0:
                    Lt = Lt0
                else:
                    Lt = lp_pool.tile([128, ccpt, shard], F8, tag="L")
                    eng = nc.gpsimd if g % 2 == 0 else nc.sync
                    eng.dma_start(
                        out=Lt[:],
                        in_=lab_ap[
                            g * ccpt * 128 : (g + 1) * ccpt * 128, :
                        ].rearrange("(cc p) i -> p cc i", p=128),
                    )
                for cc in range(ccpt):
                    ch = g * ccpt + cc
                    lw = paug[:, ch * NAUG : (ch + 1) * NAUG]
                    for h in range(nhalf):
                        i0 = h * 512
                        iw = min(512, shard - i0)
                        nc.tensor.matmul(
                            mps[0:NAUG, i0 : i0 + iw],
                            lw,
                            Lt[:, cc, i0 : i0 + iw],
                            start=(ch == 0),
                            stop=(ch == njc - 1),
                        )

            # ---------------- q / positive (overlaps main loop) ------------
            QRAW = cp.tile([128, nblk * 64], FP)
            nc.gpsimd.dma_start(out=QRAW[:], in_=q_d.ap())
            lq = cp.tile([128, nblk * 64], FP)
            nc.scalar.activation(lq[:], QRAW[:], AF.Ln)

            Pmy = cp.tile([128, nblk * 64], FP)
            nc.gpsimd.dma_start(out=Pmy[:], in_=pmy_d.ap())
            LPmy = cp.tile([128, nblk * 64], FP)
            nc.scalar.activation(LPmy[:], Pmy[:], AF.Ln)
            tsub = cp.tile([128, nblk * 64], FP)
            nc.vector.tensor_tensor(tsub[:], LPmy[:], lq[:], op=ALU.subtract)
            pos_sb = cp.tile([128, nblk], FP)
            for blk in range(nblk):
                pscr = sp.tile([128, 64], FP, tag="pscr")
                nc.vector.scalar_tensor_tensor(
                    out=pscr[:],
                    in0=Pmy[:, blk * 64 : (blk + 1) * 64],
                    scalar=rcpC,
                    in1=tsub[:, blk * 64 : (blk + 1) * 64],
                    op0=ALU.mult,
                    op1=ALU.mult,
                    accum_out=pos_sb[:, blk : blk + 1],
                )

            # ---------------- epilogue ----------------
            DpT = cp.tile([128, shard], FP)
            nc.scalar.copy(DpT[0:NAUG, :], mps[0:NAUG, :])
            updp = cp.tile([128, nblk], FP)
            da2 = cp.tile([128, nblk * 2], FP)
            for blk in range(nblk):
                tr = tr_ps.tile([128, NAUG], FP, tag="tr")
                nc.tensor.transpose(
                    tr[:],
                    DpT[0:NAUG, blk * 128 : (blk + 1) * 128],
                    ident[0:NAUG, 0:NAUG],
                )
                escr = sp.tile([128, 64], FP, tag="escr")
                nc.vector.scalar_tensor_tensor(
                    out=escr[:],
                    in0=tr[:, 0:64],
                    scalar=rcpC,
                    in1=lq[:, blk * 64 : (blk + 1) * 64],
                    op0=ALU.mult,
                    op1=ALU.mult,
                    accum_out=updp[:, blk : blk + 1],
                )
                nc.scalar.copy(da2[:, blk * 2 : (blk + 1) * 2], tr[:, 64:66])
            da_sb = cp.tile([128, nblk], FP)
            da2v = da2[:].rearrange("p (n t) -> p n t", t=2)
            nc.vector.tensor_tensor(
                da_sb[:].rearrange("p (n o) -> p n o", o=1),
                da2v[:, :, 0:1],
                da2v[:, :, 1:2],
                op=ALU.add,
            )
            neg8 = cp.tile([128, nblk], FP)
            nc.vector.scalar_tensor_tensor(
                out=neg8[:],
                in0=updp[:],
                scalar=-1.0,
                in1=da_sb[:],
                op0=ALU.mult,
                op1=ALU.add,
            )
            rec8 = cp.tile([128, nblk], FP)
            nc.vector.reciprocal(rec8[:], neg8[:])
            r8 = cp.tile([128, nblk], FP)
            nc.vector.tensor_tensor(r8[:], pos_sb[:], rec8[:], op=ALU.mult)
            out_col = cp.tile([128, 1], FP)
            nc.vector.reduce_sum(out_col[:], r8[:], axis=AX.X)
            nc.sync.dma_start(out=out_d.ap(), in_=out_col[:])

    nc.compile()
    return nc


_NC_CACHE = {}


def _get_nc(B, shard):
    key = (B, shard)
    if key not in _NC_CACHE:
        _NC_CACHE[key] = build_nc(B, shard)
    return _NC_CACHE[key]


def make_dt(labels_shard):
    """(2 - labels)^T as contiguous fp8e4m3 [B, shard]."""
    return (2.0 - labels_shard).T.astype(ml_dtypes.float8_e4m3, order="C")


def chunk_rows(arr):
    """[N, 64] fp32 -> [128, (N/128)*64]: partition pp, col n*64+c = row
    n*128+pp — the on-chip chunked layout, pre-computed on host so the
    DMA is a contiguous line-rate load."""
    n = arr.shape[0] // 128
    return np.ascontiguousarray(
        arr.reshape(n, 128, 64).transpose(1, 0, 2).reshape(128, n * 64)
    )


def make_in_maps(q, p, labels_matrix, n_cores=N_CORES):
    B = q.shape[0]
    shard = B // n_cores
    maps = []
    p_ch = chunk_rows(p)
    for k in range(n_cores):
        s = slice(k * shard, (k + 1) * shard)
        maps.append(
            {
                "q": chunk_rows(q[s]),
                "p": p_ch,
                "p_my": chunk_rows(p[s]),
                "labels": make_dt(labels_matrix[s]),
            }
        )
    return maps


def kernel(q, p, labels_matrix):
    from concourse.bass_utils import run_bass_kernel_spmd

    q = np.asarray(q, dtype=np.float32)
    p = np.asarray(p, dtype=np.float32)
    labels_matrix = np.asarray(labels_matrix, dtype=np.float32)
    B = q.shape[0]
    shard = B // N_CORES
    nc = _get_nc(B, shard)
    in_maps = make_in_maps(q, p, labels_matrix, N_CORES)
    res = run_bass_kernel_spmd(nc, in_maps, core_ids=list(range(N_CORES)))
    total = 0.0
    for r in res.results:
        total += r["out"].astype(np.float64).sum()
    return np.float32(total)



# revision 10
# speedup vs baseline: 1.2308x; 1.2308x over previous
"""Trainium2 Bass kernel for nn_DistributionLossWithLabel.

Reference computation (B=8192, C=64):
    lq = log(q); lp = log(p)
    positive[i] = mean_c p[i,c]*(lp[i,c]-lq[i,c])
    a[j]        = sum_c p[j,c]*lp[j,c] / C
    kl[i,j]     = a[j] - (lq @ p^T)[i,j] / C
    negative[i] = sum_j kl[i,j] + sum_j kl[i,j]*(1-L[i,j])
    loss        = sum_i positive[i]/negative[i]

Device reformulation (rows i sharded 8 ways; L^T shipped from host as raw
fp8e4m3 {0,1} in a per-partition-contiguous tiled layout):
    negative[i] = 2*Sa - (L@a)[i] - (1/C)*sum_c lq[i,c]*(2*Sp_c - (L@p)[i,c])
    with Sa = sum_j a[j], Sp = sum_j p[j,:] exact fp32 host constants.  The
    only O(B^2) work is M = W^T @ L^T on the TensorEngine, with fp8 weights
    W = [512*p | 3-way fp8 split of 32*a] streamed in DoubleRow perf mode
    (2 fp8 contraction rows/cycle).  The "compensated" form keeps the exact
    i-independent 2*Sp/2*Sa part in fp32, halving the fp8 quantization error.
    The 8192x8192 KL matrix never exists; the kernel is bound by reading
    L^T once (8MB/core) on two parallel HWDGE rings.
"""

import sys

if "/opt/trn_rl_repo" not in sys.path:
    sys.path.insert(0, "/opt/trn_rl_repo")

import ml_dtypes
import numpy as np

import concourse.bass as bass
import concourse.tile as tile
from concourse import bacc, mybir
from concourse.masks import make_identity

FP = mybir.dt.float32
BF = mybir.dt.bfloat16
F8 = mybir.dt.float8e4
AF = mybir.ActivationFunctionType
ALU = mybir.AluOpType
AX = mybir.AxisListType

B_FULL = 8192
C = 64
N_CORES = 8
M_W = 80          # weight columns: 64 p + 3 a-splits + 13 pad (16B-aligned)
SCALE_P = 512.0   # host scale on p columns (keeps fp8 e4m3 in normal range)
SCALE_A = 32.0    # host scale on a, and ratio between a-split columns
USE_DR = True     # DoubleRow fp8 perf mode (2 contraction rows/cycle)
NWARM = 10        # fp32 warmup matmuls to lift the PE HAM clock gate early


def _tile_plan(njc):
    """Label DMA tiles as chunk counts: small first tiles to start the MM
    stream early, then 1MB tiles for bandwidth."""
    if njc <= 8:
        return [njc]
    assert njc % 8 == 0
    plan = [4, 4] + [8] * ((njc - 8) // 8)
    return plan


def build_nc(B=B_FULL, shard=B_FULL // N_CORES, debug=False):
    assert B % 256 == 0 and shard % 128 == 0
    njc = B // 128           # 128-row j-chunks
    nblk = shard // 128      # 128-row i-blocks of this core's shard
    nhalf = (shard + 511) // 512
    plan = _tile_plan(njc)
    rcpC = 1.0 / C

    nc = bacc.Bacc("TRN2", target_bir_lowering=False, debug=debug)

    # L^T {0,1} fp8: [128, njc*shard/128] — partition pp holds, for each
    # chunk ch, the shard-wide row j=ch*128+pp contiguously (line-rate DMA).
    lab_d = nc.dram_tensor("labels", [128, njc * shard], F8, kind="ExternalInput")
    # W chunked fp8: [128, njc*M_W]; chunk ch cols: 512*p | a-splits | 0-pad
    w_d = nc.dram_tensor("wts", [128, njc * M_W], F8, kind="ExternalInput")
    # q and p_my chunked fp32 [128, nblk*64]
    q_d = nc.dram_tensor("q", [128, nblk * 64], FP, kind="ExternalInput")
    pmy_d = nc.dram_tensor("p_my", [128, nblk * 64], FP, kind="ExternalInput")
    # misc fp32 [128, 64 + nblk + 8 + 3]: cols 0:64 Sp (exact col sums of
    # p), 64:64+nblk 32*a_my, next 8 cols 64*Sa replicated, last 3 cols the
    # a-split recombination weights (1, 1/32, 1/1024)
    NM = 64 + nblk + 8 + 3
    misc_d = nc.dram_tensor("misc", [128, NM], FP, kind="ExternalInput")
    out_d = nc.dram_tensor("out", [1, 8], FP, kind="ExternalOutput")

    with tile.TileContext(nc) as tc:
        with (
            tc.tile_pool(name="const", bufs=1) as cp,
            tc.tile_pool(name="lsmall", bufs=2) as lps,
            tc.tile_pool(name="lbig", bufs=3) as lpb,
            tc.tile_pool(name="spool", bufs=2) as sp,
            tc.tile_pool(name="dsb", bufs=2) as dp,
            tc.tile_pool(name="mps_ps", bufs=1, space="PSUM") as mps_ps,
            tc.tile_pool(name="warm_ps", bufs=1, space="PSUM") as wm_ps,
            tc.tile_pool(name="tr_ps", bufs=2, space="PSUM") as tr_ps,
            tc.tile_pool(name="fin_ps", bufs=1, space="PSUM") as fin_ps,
        ):
            # ---------- identity + warmup (PE busy from ~0.3us) ----------
            ident = cp.tile([128, 128], FP)
            make_identity(nc, ident[:])
            warm = wm_ps.tile([128, 128], FP)
            for wi in range(NWARM):
                nc.tensor.matmul(warm[:], ident[:], ident[:],
                                 start=True, stop=True)

            # ---------- DMAs ----------
            # First two label tiles immediately on both HWDGE rings, then
            # host-side tensors on gpsimd (SWDGE); the Ln is emitted on the
            # scalar queue BEFORE the remaining scalar-ring label DMA issues
            # so it isn't stuck behind their buffer-backpressure waits.
            lab_ap = lab_d.ap()

            def issue_ltile(pool, tag, off, cc, eng):
                lt = pool.tile([128, cc, shard], F8, tag=tag)
                eng.dma_start(
                    out=lt[:],
                    in_=lab_ap[:, off * shard:(off + cc) * shard].rearrange(
                        "p (cc i) -> p cc i", cc=cc),
                )
                return lt

            ltiles = []
            off = 0
            for t, cc in enumerate(plan[:2]):
                pool = lps if cc < 8 else lpb
                eng = nc.sync if t % 2 == 0 else nc.scalar
                ltiles.append(
                    (issue_ltile(pool, f"L{min(cc, 8)}", off, cc, eng),
                     off, cc))
                off += cc

            W = cp.tile([128, njc, M_W], F8)
            nc.gpsimd.dma_start(out=W[:], in_=w_d.ap().rearrange(
                "p (n w) -> p n w", w=M_W))
            QRAW = cp.tile([128, nblk * 64], FP)
            nc.gpsimd.dma_start(out=QRAW[:], in_=q_d.ap())
            Pmy = cp.tile([128, nblk * 64], FP)
            nc.gpsimd.dma_start(out=Pmy[:], in_=pmy_d.ap())
            MISC = cp.tile([128, NM], FP)
            nc.gpsimd.dma_start(out=MISC[:], in_=misc_d.ap())

            lq = cp.tile([128, nblk * 64], FP)
            nc.scalar.activation(lq[:], QRAW[:], AF.Ln)

            for t, cc in enumerate(plan[2:], start=2):
                pool = lps if cc < 8 else lpb
                eng = nc.sync if t % 2 == 0 else nc.scalar
                ltiles.append(
                    (issue_ltile(pool, f"L{min(cc, 8)}", off, cc, eng),
                     off, cc))
                off += cc

            # ---------- main loop: M[m, i] += W[:,pair]^T @ L^T[:,pair] ----
            mps = mps_ps.tile([128, shard], FP)
            npair = njc // 2
            for (lt, off, cc) in ltiles:
                if USE_DR:
                    for c in range(cc // 2):
                        pr = off // 2 + c
                        lw = W[:, off + 2 * c: off + 2 * c + 2, :]
                        for h in range(nhalf):
                            i0 = h * 512
                            iw = min(512, shard - i0)
                            nc.tensor.matmul(
                                mps[0:M_W, i0:i0 + iw],
                                lw,
                                lt[:, 2 * c:2 * c + 2, i0:i0 + iw],
                                start=(pr == 0),
                                stop=(pr == npair - 1),
                                perf_mode=mybir.MatmulPerfMode.DoubleRow,
                            )
                else:
                    for c in range(cc):
                        ch = off + c
                        lw = W[:, ch, :]
                        for h in range(nhalf):
                            i0 = h * 512
                            iw = min(512, shard - i0)
                            nc.tensor.matmul(
                                mps[0:M_W, i0:i0 + iw],
                                lw,
                                lt[:, c, i0:i0 + iw],
                                start=(ch == 0),
                                stop=(ch == njc - 1),
                            )

            # ---------- during-loop work (vector/scalar idle anyway) ------
            posacc = cp.tile([128, nblk], FP)   # -(32/C) * sum_c p_my*lq
            t3acc = cp.tile([128, nblk], FP)    # (64/C) * sum_c lq*Sp
            for blk in range(nblk):
                cs = slice(blk * 64, (blk + 1) * 64)
                pscr = sp.tile([128, 64], FP, tag="pscr")
                nc.vector.scalar_tensor_tensor(
                    out=pscr[:], in0=Pmy[:, cs], scalar=-SCALE_A / C,
                    in1=lq[:, cs], op0=ALU.mult, op1=ALU.mult,
                    accum_out=posacc[:, blk:blk + 1],
                )
                tscr = sp.tile([128, 64], FP, tag="tscr")
                nc.vector.scalar_tensor_tensor(
                    out=tscr[:], in0=MISC[:, 0:64], scalar=2.0 * SCALE_A / C,
                    in1=lq[:, cs], op0=ALU.mult, op1=ALU.mult,
                    accum_out=t3acc[:, blk:blk + 1],
                )

            # ---------- epilogue: transpose M, assemble, reduce ----------
            updq = cp.tile([128, nblk], FP)     # (1/1024)*sum_c lq*M_c
            daM = cp.tile([128, nblk], FP)      # M64 + M65/32 + M66/1024
            for blk in range(nblk):
                dsb = dp.tile([128, 128], FP, tag="dsb")
                nc.scalar.copy(dsb[0:M_W, :],
                               mps[0:M_W, blk * 128:(blk + 1) * 128])
                tr = tr_ps.tile([128, M_W], FP, tag="tr")
                nc.tensor.transpose(tr[:], dsb[0:M_W, :], ident[0:M_W, 0:M_W])
                escr = sp.tile([128, 64], FP, tag="escr")
                nc.vector.scalar_tensor_tensor(
                    out=escr[:], in0=tr[:, 0:64],
                    scalar=SCALE_A / (SCALE_P * C),
                    in1=lq[:, blk * 64:(blk + 1) * 64],
                    op0=ALU.mult, op1=ALU.mult,
                    accum_out=updq[:, blk:blk + 1],
                )
                dscr = sp.tile([128, 3], FP, tag="dscr")
                nc.vector.scalar_tensor_tensor(
                    out=dscr[:], in0=tr[:, 64:67], scalar=1.0,
                    in1=MISC[:, 64 + nblk + 8:64 + nblk + 11],
                    op0=ALU.mult, op1=ALU.mult,
                    accum_out=daM[:, blk:blk + 1],
                )
            # neg32 = 64*Sa - daM - t3 + updq ; pos32 = 32*a_my + posacc
            x1 = cp.tile([128, nblk], FP)
            nc.vector.tensor_tensor(x1[:], updq[:], daM[:], op=ALU.subtract)
            x2 = cp.tile([128, nblk], FP)
            nc.vector.tensor_tensor(x2[:], x1[:], t3acc[:], op=ALU.subtract)
            neg32 = cp.tile([128, nblk], FP)
            nc.vector.tensor_tensor(
                neg32[:], x2[:], MISC[:, 64 + nblk:64 + nblk + nblk],
                op=ALU.add)
            pos32 = cp.tile([128, nblk], FP)
            nc.vector.tensor_tensor(
                pos32[:], posacc[:], MISC[:, 64:64 + nblk], op=ALU.add)
            rec = cp.tile([128, nblk], FP)
            nc.vector.reciprocal(rec[:], neg32[:])
            r8 = cp.tile([128, nblk], FP)
            nc.vector.tensor_tensor(r8[:], pos32[:], rec[:], op=ALU.mult)
            # partition-reduce via ones-matmul -> single-descriptor out DMA
            ones = cp.tile([128, 1], FP)
            nc.gpsimd.memset(ones[:], 1.0)
            fin = fin_ps.tile([1, 8], FP)
            nc.tensor.matmul(fin[0:1, 0:nblk], ones[:], r8[:],
                             start=True, stop=True)
            fin_sb = cp.tile([1, 8], FP)
            if nblk < 8:
                nc.gpsimd.memset(fin_sb[:], 0.0)
            nc.scalar.copy(fin_sb[:, 0:nblk], fin[0:1, 0:nblk])
            nc.sync.dma_start(out=out_d.ap(), in_=fin_sb[:])

    nc.compile()
    return nc


_NC_CACHE = {}


def _get_nc(B, shard):
    key = (B, shard)
    if key not in _NC_CACHE:
        _NC_CACHE[key] = build_nc(B, shard)
    return _NC_CACHE[key]


def chunk_rows(arr, w=64):
    """[N, w] -> [128, (N/128)*w], partition pp col n*w+c = row n*128+pp."""
    n = arr.shape[0] // 128
    return np.ascontiguousarray(
        arr.reshape(n, 128, w).transpose(1, 0, 2).reshape(128, n * w)
    )


def _f8(x):
    return x.astype(ml_dtypes.float8_e4m3)


def make_in_maps(q, p, labels_matrix, n_cores=N_CORES):
    B, nC = q.shape
    shard = B // n_cores
    njc = B // 128
    nblk = shard // 128

    lp = np.log(p)
    a = (p * lp).sum(axis=1, dtype=np.float64).astype(np.float32) / nC
    Sp = p.sum(axis=0, dtype=np.float64).astype(np.float32)
    Sa = np.float32(a.sum(dtype=np.float64))

    # fp8 weight block W [B, M_W]: 512*p | 3-way split of 32*a | zero pad
    Wf = np.zeros((B, M_W), dtype=ml_dtypes.float8_e4m3)
    Wf[:, 0:nC] = _f8(p * SCALE_P)
    v0 = SCALE_A * a
    c64 = _f8(v0)
    r1 = v0 - c64.astype(np.float32)
    c65 = _f8(SCALE_A * r1)
    r2 = SCALE_A * r1 - c65.astype(np.float32)
    c66 = _f8(SCALE_A * r2)
    Wf[:, nC] = c64
    Wf[:, nC + 1] = c65
    Wf[:, nC + 2] = c66
    w_ch = np.ascontiguousarray(
        Wf.reshape(njc, 128, M_W).transpose(1, 0, 2).reshape(128, njc * M_W)
    )

    maps = []
    for k in range(n_cores):
        s = slice(k * shard, (k + 1) * shard)
        # L^T chunk layout: [128, njc*shard] fp8, partition pp chunk ch =
        # row j=ch*128+pp of L^T = column j of L_shard, contiguous in i
        Lt = _f8(labels_matrix[s].T)                       # [B, shard]
        lab = np.ascontiguousarray(
            Lt.reshape(njc, 128, shard).transpose(1, 0, 2).reshape(
                128, njc * shard)
        )
        misc = np.zeros((128, 64 + nblk + 8 + 3), dtype=np.float32)
        misc[:, 0:64] = Sp[None, :]
        misc[:, 64:64 + nblk] = SCALE_A * chunk_rows(a[s].reshape(shard, 1), 1)
        misc[:, 64 + nblk:64 + nblk + 8] = 2.0 * SCALE_A * Sa
        misc[:, 64 + nblk + 8] = 1.0
        misc[:, 64 + nblk + 9] = 1.0 / SCALE_A
        misc[:, 64 + nblk + 10] = 1.0 / SCALE_A ** 2
        maps.append(
            {
                "labels": lab,
                "wts": w_ch,
                "q": chunk_rows(q[s]),
                "p_my": chunk_rows(p[s]),
                "misc": misc,
            }
        )
    return maps


def kernel(q, p, labels_matrix):
    from concourse.bass_utils import run_bass_kernel_spmd

    q = np.asarray(q, dtype=np.float32)
    p = np.asarray(p, dtype=np.float32)
    labels_matrix = np.asarray(labels_matrix, dtype=np.float32)
    B = q.shape[0]
    shard = B // N_CORES
    nc = _get_nc(B, shard)
    in_maps = make_in_maps(q, p, labels_matrix, N_CORES)
    res = run_bass_kernel_spmd(nc, in_maps, core_ids=list(range(N_CORES)))
    total = 0.0
    for r in res.results:
        total += r["out"].astype(np.float64).sum()
    return np.float32(total)


# revision 15
# speedup vs baseline: 1.5284x; 1.2417x over previous
"""Trainium2 Bass kernel for nn_DistributionLossWithLabel.

Reference computation (B=8192, C=64):
    lq = log(q); lp = log(p)
    positive[i] = mean_c p[i,c]*(lp[i,c]-lq[i,c])
    a[j]        = sum_c p[j,c]*lp[j,c] / C
    kl[i,j]     = a[j] - (lq @ p^T)[i,j] / C
    negative[i] = sum_j kl[i,j] + sum_j kl[i,j]*(1-L[i,j])
    loss        = sum_i positive[i]/negative[i]

Device reformulation (rows i sharded 8 ways; L^T shipped from host as raw
fp8e4m3 {0,1} in a per-partition-contiguous tiled layout):
    negative[i] = 2*Sa - (L@a)[i] - (1/C)*sum_c lq[i,c]*(2*Sp_c - (L@p)[i,c])
    with Sa = sum_j a[j], Sp = sum_j p[j,:] exact fp32 host constants.  The
    only O(B^2) work is M = W^T @ L^T on the TensorEngine, with fp8 weights
    W = [512*p | 3-way fp8 split of 32*a] streamed in DoubleRow perf mode
    (2 fp8 contraction rows/cycle).  The "compensated" form keeps the exact
    i-independent 2*Sp/2*Sa part in fp32, halving the fp8 quantization error.
    The 8192x8192 KL matrix never exists; the kernel is bound by reading
    L^T once (8MB/core) on two parallel HWDGE rings.
"""

import sys

if "/opt/trn_rl_repo" not in sys.path:
    sys.path.insert(0, "/opt/trn_rl_repo")

import ml_dtypes
import numpy as np

import concourse.bass as bass
import concourse.tile as tile
from concourse import bacc, mybir
from concourse.masks import make_identity

FP = mybir.dt.float32
BF = mybir.dt.bfloat16
F8 = mybir.dt.float8e4
AF = mybir.ActivationFunctionType
ALU = mybir.AluOpType
AX = mybir.AxisListType

B_FULL = 8192
C = 64
N_CORES = 8
M_W = 80          # weight columns: 64 p + 3 a-splits + 13 pad (16B-aligned)
SCALE_P = 512.0   # host scale on p columns (keeps fp8 e4m3 in normal range)
SCALE_A = 32.0    # host scale on a, and ratio between a-split columns
USE_DR = True     # DoubleRow fp8 perf mode (2 contraction rows/cycle)
NWARM = 7         # fp32 warmup matmuls to lift the PE HAM clock gate early


def _tile_plan(njc):
    """Label DMA tiles as chunk counts: small first tiles to start the MM
    stream early, then 1MB tiles for bandwidth."""
    if njc <= 8:
        return [njc]
    assert njc % 8 == 0
    plan = [2, 2, 4] + [8] * ((njc - 8) // 8)
    return plan


def build_nc(B=B_FULL, shard=B_FULL // N_CORES, debug=False):
    assert B % 256 == 0 and shard % 128 == 0
    njc = B // 128           # 128-row j-chunks
    nblk = shard // 128      # 128-row i-blocks of this core's shard
    nhalf = (shard + 511) // 512
    plan = _tile_plan(njc)
    rcpC = 1.0 / C

    nc = bacc.Bacc("TRN2", target_bir_lowering=False, debug=debug)

    # L^T {0,1} fp8: [128, njc*shard/128] — partition pp holds, for each
    # chunk ch, the shard-wide row j=ch*128+pp contiguously (line-rate DMA).
    lab_d = nc.dram_tensor("labels", [128, njc * shard], F8, kind="ExternalInput")
    # W chunked fp8: [128, njc*M_W]; chunk ch cols: 512*p | a-splits | 0-pad
    w_d = nc.dram_tensor("wts", [128, njc * M_W], F8, kind="ExternalInput")
    # q and p_my chunked fp32 [128, nblk*64]
    q_d = nc.dram_tensor("q", [128, nblk * 64], FP, kind="ExternalInput")
    pmy_d = nc.dram_tensor("p_my", [128, nblk * 64], FP, kind="ExternalInput")
    # misc fp32 [128, 64 + nblk + 8 + 3]: cols 0:64 Sp (exact col sums of
    # p), 64:64+nblk 32*a_my, next 8 cols 64*Sa replicated, last 3 cols the
    # a-split recombination weights (1, 1/32, 1/1024)
    NM = 64 + nblk + 8 + 3
    misc_d = nc.dram_tensor("misc", [128, NM], FP, kind="ExternalInput")
    out_d = nc.dram_tensor("out", [1, 8], FP, kind="ExternalOutput")

    with tile.TileContext(nc) as tc:
        with (
            tc.tile_pool(name="const", bufs=1) as cp,
            tc.tile_pool(name="lsmall", bufs=2) as lps,
            tc.tile_pool(name="lmid", bufs=1) as lpm,
            tc.tile_pool(name="lbig", bufs=4) as lpb,
            tc.tile_pool(name="spool", bufs=2) as sp,
            tc.tile_pool(name="dsb", bufs=2) as dp,
            tc.tile_pool(name="mps_ps", bufs=1, space="PSUM") as mps_ps,
            tc.tile_pool(name="warm_ps", bufs=1, space="PSUM") as wm_ps,
            tc.tile_pool(name="tr_ps", bufs=2, space="PSUM") as tr_ps,
            tc.tile_pool(name="fin_ps", bufs=1, space="PSUM") as fin_ps,
        ):
            # ---------- identity + warmup (PE busy from ~0.3us) ----------
            ident = cp.tile([128, 128], FP)
            make_identity(nc, ident[:])
            warm = wm_ps.tile([128, 128], FP)
            for wi in range(NWARM):
                nc.tensor.matmul(warm[:], ident[:], ident[:],
                                 start=True, stop=True)

            # ---------- DMAs ----------
            # W first on the sync HWDGE ring as a flat per-partition-
            # contiguous transfer (the first matmul needs it), then label
            # tiles alternating across the sync and scalar HWDGE rings.
            # Small host tensors ride the gpsimd (SWDGE) ring.
            Wf = cp.tile([128, njc * M_W], F8)
            nc.sync.dma_start(out=Wf[:], in_=w_d.ap())
            W = Wf[:].rearrange("p (n w) -> p n w", w=M_W)

            lab_ap = lab_d.ap()
            ltiles = []
            off = 0
            for t, cc in enumerate(plan):
                pool = {2: lps, 4: lpm}.get(cc, lpb)
                lt = pool.tile([128, cc, shard], F8, tag=f"L{min(cc, 8)}")
                eng = nc.sync if t % 2 == 0 else nc.scalar
                eng.dma_start(
                    out=lt[:],
                    in_=lab_ap[:, off * shard:(off + cc) * shard].rearrange(
                        "p (cc i) -> p cc i", cc=cc),
                )
                ltiles.append((lt, off, cc))
                off += cc

            QRAW = cp.tile([128, nblk * 64], FP)
            nc.gpsimd.dma_start(out=QRAW[:], in_=q_d.ap())
            Pmy = cp.tile([128, nblk * 64], FP)
            nc.gpsimd.dma_start(out=Pmy[:], in_=pmy_d.ap())
            MISC = cp.tile([128, NM], FP)
            nc.gpsimd.dma_start(out=MISC[:], in_=misc_d.ap())

            lq = cp.tile([128, nblk * 64], FP)
            nc.scalar.activation(lq[:], QRAW[:], AF.Ln)

            # ---------- main loop: M[m, i] += W[:,pair]^T @ L^T[:,pair] ----
            mps = mps_ps.tile([128, shard], FP)
            npair = njc // 2
            for (lt, off, cc) in ltiles:
                if USE_DR:
                    for c in range(cc // 2):
                        pr = off // 2 + c
                        lw = W[:, off + 2 * c: off + 2 * c + 2, :]
                        for h in range(nhalf):
                            i0 = h * 512
                            iw = min(512, shard - i0)
                            nc.tensor.matmul(
                                mps[0:M_W, i0:i0 + iw],
                                lw,
                                lt[:, 2 * c:2 * c + 2, i0:i0 + iw],
                                start=(pr == 0),
                                stop=(pr == npair - 1),
                                perf_mode=mybir.MatmulPerfMode.DoubleRow,
                            )
                else:
                    for c in range(cc):
                        ch = off + c
                        lw = Wf[:, ch * M_W:(ch + 1) * M_W]
                        for h in range(nhalf):
                            i0 = h * 512
                            iw = min(512, shard - i0)
                            nc.tensor.matmul(
                                mps[0:M_W, i0:i0 + iw],
                                lw,
                                lt[:, c, i0:i0 + iw],
                                start=(ch == 0),
                                stop=(ch == njc - 1),
                            )

            # ---------- during-loop work (vector/scalar idle anyway) ------
            posacc = cp.tile([128, nblk], FP)   # -(32/C) * sum_c p_my*lq
            t3acc = cp.tile([128, nblk], FP)    # (64/C) * sum_c lq*Sp
            for blk in range(nblk):
                cs = slice(blk * 64, (blk + 1) * 64)
                pscr = sp.tile([128, 64], FP, tag="pscr")
                nc.vector.scalar_tensor_tensor(
                    out=pscr[:], in0=Pmy[:, cs], scalar=-SCALE_A / C,
                    in1=lq[:, cs], op0=ALU.mult, op1=ALU.mult,
                    accum_out=posacc[:, blk:blk + 1],
                )
                tscr = sp.tile([128, 64], FP, tag="tscr")
                nc.vector.scalar_tensor_tensor(
                    out=tscr[:], in0=MISC[:, 0:64], scalar=2.0 * SCALE_A / C,
                    in1=lq[:, cs], op0=ALU.mult, op1=ALU.mult,
                    accum_out=t3acc[:, blk:blk + 1],
                )

            # ---------- epilogue: transpose M, assemble, reduce ----------
            updq = cp.tile([128, nblk], FP)     # (1/1024)*sum_c lq*M_c
            daM = cp.tile([128, nblk], FP)      # M64 + M65/32 + M66/1024
            for blk in range(nblk):
                dsb = dp.tile([128, 128], FP, tag="dsb")
                nc.scalar.copy(dsb[0:M_W, :],
                               mps[0:M_W, blk * 128:(blk + 1) * 128])
                tr = tr_ps.tile([128, M_W], FP, tag="tr")
                nc.tensor.transpose(tr[:], dsb[0:M_W, :], ident[0:M_W, 0:M_W])
                escr = sp.tile([128, 64], FP, tag="escr")
                nc.vector.scalar_tensor_tensor(
                    out=escr[:], in0=tr[:, 0:64],
                    scalar=SCALE_A / (SCALE_P * C),
                    in1=lq[:, blk * 64:(blk + 1) * 64],
                    op0=ALU.mult, op1=ALU.mult,
                    accum_out=updq[:, blk:blk + 1],
                )
                dscr = sp.tile([128, 3], FP, tag="dscr")
                nc.vector.scalar_tensor_tensor(
                    out=dscr[:], in0=tr[:, 64:67], scalar=1.0,
                    in1=MISC[:, 64 + nblk + 8:64 + nblk + 11],
                    op0=ALU.mult, op1=ALU.mult,
                    accum_out=daM[:, blk:blk + 1],
                )
            # neg32 = 64*Sa - daM - t3 + updq ; pos32 = 32*a_my + posacc
            x1 = cp.tile([128, nblk], FP)
            nc.vector.tensor_tensor(x1[:], updq[:], daM[:], op=ALU.subtract)
            x2 = cp.tile([128, nblk], FP)
            nc.vector.tensor_tensor(x2[:], x1[:], t3acc[:], op=ALU.subtract)
            neg32 = cp.tile([128, nblk], FP)
            nc.vector.tensor_tensor(
                neg32[:], x2[:], MISC[:, 64 + nblk:64 + nblk + nblk],
                op=ALU.add)
            pos32 = cp.tile([128, nblk], FP)
            nc.vector.tensor_tensor(
                pos32[:], posacc[:], MISC[:, 64:64 + nblk], op=ALU.add)
            rec = cp.tile([128, nblk], FP)
            nc.vector.reciprocal(rec[:], neg32[:])
            r8 = cp.tile([128, nblk], FP)
            nc.vector.tensor_tensor(r8[:], pos32[:], rec[:], op=ALU.mult)
            # partition-reduce via ones-matmul -> single-descriptor out DMA
            ones = cp.tile([128, 1], FP)
            nc.gpsimd.memset(ones[:], 1.0)
            fin = fin_ps.tile([1, 8], FP)
            nc.tensor.matmul(fin[0:1, 0:nblk], ones[:], r8[:],
                             start=True, stop=True)
            fin_sb = cp.tile([1, 8], FP)
            if nblk < 8:
                nc.gpsimd.memset(fin_sb[:], 0.0)
            nc.scalar.copy(fin_sb[:, 0:nblk], fin[0:1, 0:nblk])
            nc.sync.dma_start(out=out_d.ap(), in_=fin_sb[:])

    nc.compile()
    return nc


_NC_CACHE = {}


def _get_nc(B, shard):
    key = (B, shard)
    if key not in _NC_CACHE:
        _NC_CACHE[key] = build_nc(B, shard)
    return _NC_CACHE[key]


def chunk_rows(arr, w=64):
    """[N, w] -> [128, (N/128)*w], partition pp col n*w+c = row n*128+pp."""
    n = arr.shape[0] // 128
    return np.ascontiguousarray(
        arr.reshape(n, 128, w).transpose(1, 0, 2).reshape(128, n * w)
    )


def _f8(x):
    return x.astype(ml_dtypes.float8_e4m3)


def make_in_maps(q, p, labels_matrix, n_cores=N_CORES):
    B, nC = q.shape
    shard = B // n_cores
    njc = B // 128
    nblk = shard // 128

    lp = np.log(p)
    a = (p * lp).sum(axis=1, dtype=np.float64).astype(np.float32) / nC
    Sp = p.sum(axis=0, dtype=np.float64).astype(np.float32)
    Sa = np.float32(a.sum(dtype=np.float64))

    # fp8 weight block W [B, M_W]: 512*p | 3-way split of 32*a | zero pad
    Wf = np.zeros((B, M_W), dtype=ml_dtypes.float8_e4m3)
    Wf[:, 0:nC] = _f8(p * SCALE_P)
    v0 = SCALE_A * a
    c64 = _f8(v0)
    r1 = v0 - c64.astype(np.float32)
    c65 = _f8(SCALE_A * r1)
    r2 = SCALE_A * r1 - c65.astype(np.float32)
    c66 = _f8(SCALE_A * r2)
    Wf[:, nC] = c64
    Wf[:, nC + 1] = c65
    Wf[:, nC + 2] = c66
    w_ch = np.ascontiguousarray(
        Wf.reshape(njc, 128, M_W).transpose(1, 0, 2).reshape(128, njc * M_W)
    )

    maps = []
    for k in range(n_cores):
        s = slice(k * shard, (k + 1) * shard)
        # L^T chunk layout: [128, njc*shard] fp8, partition pp chunk ch =
        # row j=ch*128+pp of L^T = column j of L_shard, contiguous in i
        Lt = _f8(labels_matrix[s].T)                       # [B, shard]
        lab = np.ascontiguousarray(
            Lt.reshape(njc, 128, shard).transpose(1, 0, 2).reshape(
                128, njc * shard)
        )
        misc = np.zeros((128, 64 + nblk + 8 + 3), dtype=np.float32)
        misc[:, 0:64] = Sp[None, :]
        misc[:, 64:64 + nblk] = SCALE_A * chunk_rows(a[s].reshape(shard, 1), 1)
        misc[:, 64 + nblk:64 + nblk + 8] = 2.0 * SCALE_A * Sa
        misc[:, 64 + nblk + 8] = 1.0
        misc[:, 64 + nblk + 9] = 1.0 / SCALE_A
        misc[:, 64 + nblk + 10] = 1.0 / SCALE_A ** 2
        maps.append(
            {
                "labels": lab,
                "wts": w_ch,
                "q": chunk_rows(q[s]),
                "p_my": chunk_rows(p[s]),
                "misc": misc,
            }
        )
    return maps


def kernel(q, p, labels_matrix):
    from concourse.bass_utils import run_bass_kernel_spmd

    q = np.asarray(q, dtype=np.float32)
    p = np.asarray(p, dtype=np.float32)
    labels_matrix = np.asarray(labels_matrix, dtype=np.float32)
    B = q.shape[0]
    shard = B // N_CORES
    nc = _get_nc(B, shard)
    in_maps = make_in_maps(q, p, labels_matrix, N_CORES)
    res = run_bass_kernel_spmd(nc, in_maps, core_ids=list(range(N_CORES)))
    total = 0.0
    for r in res.results:
        total += r["out"].astype(np.float64).sum()
    return np.float32(total)
